# revision 34
# baseline (speedup 1.0000x reference)
"""Trainium2 Bass kernel: 16-head MHA (B=2, T=2048, D=1024), head-TP over 8 cores.

Per core c: heads 2c, 2c+1 (128 channels). Device computes x@Wqkv(+b) for its
head slice, scoresT=K@Q^T (scale folded into Wq), exp via ACT, P@V with an
appended ones-column producing the softmax denominator for free, normalize,
then partial proj = attn_c @ Wproj[c-slice]. Host sums the 8 partials + b_proj
(+ bv@Wproj, since the V bias passes through softmax as a constant).

The kernel is ACT(exp)-bound in steady state (~1.1us per key-chunk), so all
qkv projection work is JIT-scheduled into the attention loop's PE slack:
 - no separate qkv phase; attention for (b0,qc0) starts as soon as
   kT(batch0)+qT(qc0) and the first V chunks exist
 - each batch's qc0 carries its own late V chunks + second K half-group;
   q for qc+1 is emitted at iter 13 of qc; batch1's first K group, qc0
   queries and first V chunks ride in batch0's later qcs
 - proj matmuls of qc are spread into the next qc's early iterations
 - score matmuls (K=64) run concurrently on PE row-groups via tile_position
 - PE warm-up matmul stream covers the initial DMA window (HAM ramp)
 - ACT does exp only: q/k bias adds live on DVE, V bias folded into host
   b_proj; PSUM->SBUF copies on DVE (GpSimd cannot read PSUM)
"""

import numpy as np
import ml_dtypes
from contextlib import ExitStack

B, T, C = 2, 2048, 1024
H, DH = 16, 64
NCORES = 8
CH = 128               # channels per core = 2 heads
NTOK = B * T           # 4096
NKC = T // 128         # 16 key chunks per batch
NQC = T // 512         # 4 query chunks per batch
SCALE = DH ** -0.5

_CACHE = {}


def _build():
    import concourse.bass as bass  # noqa: F401
    import concourse.bacc as bacc
    import concourse.mybir as mybir
    import concourse.tile as tile

    f32 = mybir.dt.float32
    bf16 = mybir.dt.bfloat16
    EXP = mybir.ActivationFunctionType.Exp

    # Bacc (not Bass): its compile() runs move_matmul_waits_to_ldweights +
    # generate_event_semaphores, without which walrus rejects matmuls
    # carrying 2 sync waits ("Too many sync wait commands").
    nc = bacc.Bacc("TRN2", target_bir_lowering=False, debug=False)
    xT_d = nc.declare_dram_parameter("xT", [C, NTOK], bf16, isOutput=False)
    wq_d = nc.declare_dram_parameter("wq", [128, C], bf16, isOutput=False)
    wk_d = nc.declare_dram_parameter("wk", [128, C], bf16, isOutput=False)
    wv_d = nc.declare_dram_parameter("wv", [128, C], bf16, isOutput=False)
    wp_d = nc.declare_dram_parameter("wp", [CH, C], bf16, isOutput=False)
    bqc_d = nc.declare_dram_parameter("bqc", [CH, 1], f32, isOutput=False)
    bkc_d = nc.declare_dram_parameter("bkc", [CH, 1], f32, isOutput=False)
    out_d = nc.declare_dram_parameter("out", [NTOK, C], bf16, isOutput=True)

    with tile.TileContext(nc) as tc, ExitStack() as ctx:
        ep = ctx.enter_context

        # ---------------- persistent SBUF ----------------
        xT_pool = ep(tc.tile_pool(name="xT", bufs=8))
        xT_sb = [xT_pool.tile([128, NTOK], bf16, name=f"xT{k}", tag="xT") for k in range(8)]
        w_pool = ep(tc.tile_pool(name="w", bufs=4))
        wq_sb = w_pool.tile([128, C], bf16, tag="wq")
        wk_sb = w_pool.tile([128, C], bf16, tag="wk")
        wv_sb = w_pool.tile([128, C], bf16, tag="wv")
        wp_sb = w_pool.tile([CH, C], bf16, tag="wp")
        b_pool = ep(tc.tile_pool(name="bias", bufs=1))
        bqc_sb = b_pool.tile([CH, 1], f32, tag="bqc")
        bkc_sb = b_pool.tile([CH, 1], f32, tag="bkc")
        warm_pool = ep(tc.tile_pool(name="warm", bufs=1))
        warm_sb = warm_pool.tile([128, 256], bf16, tag="warm")
        qk_pool = ep(tc.tile_pool(name="qk", bufs=2))
        qT_sb = qk_pool.tile([CH, NTOK], bf16, tag="qT")
        kT_sb = qk_pool.tile([CH, NTOK], bf16, tag="kT")
        v_pool = ep(tc.tile_pool(name="v", bufs=1))
        # per key-chunk, per head: [ones 0:64 | vals 64:128]. The ones cols
        # make each head's PV matmul replicate the softmax denominator onto
        # output partitions 0:64 for free (the custom-DVE recip ignores AP
        # partition bases, so denoms must sit at partitions 0:64). Values
        # arrive via DMA-transpose (one [64ch x 512key] transpose per head
        # per 512-token group) from the vT staging tiles.
        v_sb = v_pool.tile([128, B * NKC, 2, 128], bf16, name="v", tag="v")
        vt_pool = ep(tc.tile_pool(name="vt", bufs=2))
        attn_pool = ep(tc.tile_pool(name="attn", bufs=2))
        attnT = [attn_pool.tile([CH, T], bf16, name=f"attnT{b}", tag="attnT") for b in range(B)]
        exp_pool = ep(tc.tile_pool(name="exp", bufs=4))
        bc_pool = ep(tc.tile_pool(name="bcsb", bufs=1))
        out_pool = ep(tc.tile_pool(name="outsb", bufs=4))

        # ---------------- load inputs ----------------
        # order = consumption order of the JIT schedule: wk + x(t0) feed the
        # very first K half-group, then wq (q qc0), wv (first V chunks),
        # x(t1) for the second K half, then the rest.
        def xchunk(q, k, t):
            q.dma_start(
                xT_sb[k][:, t * 512:(t + 1) * 512],
                xT_d[k * 128:(k + 1) * 128, t * 512:(t + 1) * 512])

        # warm-up feed tile first: tiny gpsimd memset with no deps, so the
        # PE warm-up matmuls can start the moment the preamble ends
        nc.gpsimd.memset(warm_sb[:], 1.0)

        nc.gpsimd.dma_start(wk_sb[:, 0:512], wk_d[:, 0:512])
        nc.sync.dma_start(wk_sb[:, 512:1024], wk_d[:, 512:1024])
        for k in range(8):
            xchunk((nc.sync, nc.gpsimd, nc.scalar)[k % 3], k, 0)
        nc.gpsimd.dma_start(wq_sb[:, 0:512], wq_d[:, 0:512])
        nc.sync.dma_start(wq_sb[:, 512:1024], wq_d[:, 512:1024])
        nc.scalar.dma_start(wv_sb[:], wv_d[:])
        nc.gpsimd.dma_start(bkc_sb[:], bkc_d[:])
        nc.gpsimd.dma_start(bqc_sb[:], bqc_d[:])
        for k in range(8):
            xchunk((nc.sync, nc.gpsimd, nc.scalar)[k % 3], k, 1)
        nc.sync.dma_start(wp_sb[:], wp_d[:])
        for t in range(2, 8):
            for k in range(8):
                q = nc.sync if k % 2 == 0 else nc.gpsimd
                xchunk(q, k, t)
        # ones columns of the v tile (value cols written by DMA-transpose)
        nc.vector.memset(v_sb[:, :, :, 0:64], 1.0)

        # ---------------- PE warm-up ----------------
        # The HAM clock gate keeps PE at 1.2 GHz until ~3.4us of sustained
        # activity. Run junk matmuls on memset data during the initial DMA
        # window so the real matmuls start at 2.4 GHz.
        with tc.tile_pool(name="warm_ps", bufs=1, space="PSUM") as warm_psp:
            wps = warm_psp.tile([64, 256], f32, name="warm_ps", tag="warm_ps")
            for _ in range(28):
                nc.tensor.matmul(wps[:], lhsT=warm_sb[:, 0:64],
                                 rhs=warm_sb[:], start=True, stop=True)

        # ---------------- attention with JIT qkv ----------------
        with tc.tile_pool(name="scores_ps", bufs=2, space="PSUM") as scores_ps, \
             tc.tile_pool(name="pv_ps", bufs=2, space="PSUM") as pv_ps, \
             tc.tile_pool(name="proj_ps", bufs=2, space="PSUM") as proj_ps:

            def emit_qk_half(w_sb, bias_col, dst, ts):
                """[CH,512] q or k projection of 512-token slice ts (0..7)"""
                ps = proj_ps.tile([128, 512], f32, name="qk_ps", tag="pj")
                sl = slice(ts * 512, (ts + 1) * 512)
                for k in range(8):
                    nc.tensor.matmul(
                        ps[:], lhsT=w_sb[:, k * 128:(k + 1) * 128],
                        rhs=xT_sb[k][:, sl], start=(k == 0), stop=(k == 7))
                nc.vector.tensor_scalar_add(dst[:, sl], ps[:], bias_col[:])

            def emit_vT_group(g):
                """v for 512-token group g (0..7), computed transposed
                [128 ch, 512 tok] with LDW-hidden N=512 matmuls, then
                DMA-transposed into the [keys, ch] layout PV needs."""
                ps = proj_ps.tile([128, 512], f32, name="vt_ps", tag="pj")
                sl = slice(g * 512, (g + 1) * 512)
                for k in range(8):
                    nc.tensor.matmul(
                        ps[:], lhsT=wv_sb[:, k * 128:(k + 1) * 128],
                        rhs=xT_sb[k][:, sl], start=(k == 0), stop=(k == 7))
                vt = vt_pool.tile([128, 512], bf16, name="vt_sb", tag="vt")
                nc.vector.tensor_copy(vt[:], ps[:])
                # [64 ch, 512 keys] -> [128 keys, 4 chunks, 64 ch] per head
                # (3D out: the middle dim extends the transposed key axis).
                # Early groups go on the scalar queue: sync's sequencer is
                # still working through the x-chunk load issues then, and a
                # transpose stuck behind those stalls the first PV matmuls.
                cs = slice(g * 4, (g + 1) * 4)
                oq = nc.scalar if g < 4 else nc.sync
                oq.dma_start_transpose(v_sb[:, cs, 0, 64:128], vt[0:64, :])
                oq.dma_start_transpose(v_sb[:, cs, 1, 64:128], vt[64:128, :])

            pending_proj = []

            def emit_proj_tc(b, tci, last=False):
                """proj partial for one 128-token chunk: out += attn @ Wp_c"""
                osb = out_pool.tile([128, 1024], bf16, name="out_sb", tag="out_sb")
                for ncol in range(2):
                    pps = proj_ps.tile([128, 512], f32, name="proj_ps", tag="pj")
                    nc.tensor.matmul(
                        pps[:],
                        lhsT=attnT[b][:, tci * 128:(tci + 1) * 128],
                        rhs=wp_sb[:, ncol * 512:(ncol + 1) * 512],
                        start=True, stop=True)
                    # ACT is idle at the very end (all exp done) — use it there
                    if last and ncol == 1:
                        nc.scalar.copy(osb[:, ncol * 512:(ncol + 1) * 512], pps[:])
                    else:
                        nc.vector.tensor_copy(
                            osb[:, ncol * 512:(ncol + 1) * 512], pps[:])
                if last:
                    oq = (nc.gpsimd, nc.sync, nc.scalar, nc.gpsimd)[tci % 4]
                else:
                    oq = nc.gpsimd if tci % 2 == 0 else nc.sync
                oq.dma_start(
                    out_d[b * T + tci * 128: b * T + (tci + 1) * 128, :], osb[:])

            def fillers(b, qc, kc):
                """JIT qkv work slotted into PE slack of iteration (b,qc,kc)"""
                if qc == 0:
                    # own V groups + remaining K halves + next-qc queries
                    if kc in (0, 4, 8):
                        emit_vT_group(b * 4 + 1 + kc // 4)
                    elif b == 0 and kc in (1, 3, 5):
                        emit_qk_half(wk_sb, bkc_sb, kT_sb, (kc + 1) // 2)
                    elif b == 1 and kc in (1, 3):
                        emit_qk_half(wk_sb, bkc_sb, kT_sb, 6 + (kc - 1) // 2)
                    elif kc == 13:
                        emit_qk_half(wq_sb, bqc_sb, qT_sb, b * 4 + 1)
                elif qc == 1:
                    if kc == 13:
                        emit_qk_half(wq_sb, bqc_sb, qT_sb, b * 4 + 2)
                elif qc == 2:
                    if b == 0 and kc == 4:
                        emit_qk_half(wk_sb, bkc_sb, kT_sb, 4)
                    elif b == 0 and kc == 8:
                        emit_qk_half(wk_sb, bkc_sb, kT_sb, 5)
                    elif kc == 13:
                        emit_qk_half(wq_sb, bqc_sb, qT_sb, b * 4 + 3)
                elif qc == 3 and b == 0:
                    # batch1 head start: first V group + qc0 queries
                    if kc == 0:
                        emit_vT_group(4)
                    elif kc == 13:
                        emit_qk_half(wq_sb, bqc_sb, qT_sb, 4)

            # pre-phase: minimal deps for (b0,qc0) — K(t0), q(qc0), V grp 0
            emit_qk_half(wk_sb, bkc_sb, kT_sb, 0)
            emit_qk_half(wq_sb, bqc_sb, qT_sb, 0)
            emit_vT_group(0)

            for b in range(B):
                for qc in range(NQC):
                    q_sl = slice(b * T + qc * 512, b * T + (qc + 1) * 512)
                    pv = [pv_ps.tile([128, 512], f32, name=f"pv{h}", tag="pv") for h in range(2)]
                    exp_tiles = {}

                    def emit_scores(kc):
                        sc = scores_ps.tile([128, 1024], f32, name="sc_ps", tag="ps")
                        k_sl = slice(b * T + kc * 128, b * T + (kc + 1) * 128)
                        # the two heads occupy PE row-groups 0-63 / 64-127 and
                        # different PSUM banks -> they execute concurrently
                        for h in range(2):
                            nc.tensor.matmul(
                                sc[:, h * 512:(h + 1) * 512],
                                lhsT=kT_sb[h * 64:(h + 1) * 64, k_sl],
                                rhs=qT_sb[h * 64:(h + 1) * 64, q_sl],
                                start=True, stop=True,
                                tile_position=(h * 64, 0))
                        ex = exp_pool.tile([128, 1024], bf16, name="exp_sb", tag="exp_sb")
                        nc.scalar.activation(ex[:], sc[:], EXP)
                        exp_tiles[kc] = ex

                    def emit_pv(kc):
                        gkc = b * NKC + kc
                        ex = exp_tiles.pop(kc)
                        for h in range(2):
                            nc.tensor.matmul(
                                pv[h][:],
                                lhsT=v_sb[:, gkc, h, :],
                                rhs=ex[:, h * 512:(h + 1) * 512],
                                start=(kc == 0), stop=(kc == NKC - 1),
                                skip_group_check=True)

                    # software-pipelined: scores run 3 ahead of PV (so the PV
                    # LDWEIGHTS' exp-semaphore wait resolves early and the
                    # weight load hides under preceding matmuls); qkv JIT
                    # jobs + previous qc's proj slotted into the ACT-bound gaps
                    for kc in range(NKC):
                        emit_scores(kc)
                        if kc >= 3:
                            emit_pv(kc - 3)
                        if kc in (2, 6, 10, 14) and pending_proj:
                            emit_proj_tc(*pending_proj.pop(0))
                        fillers(b, qc, kc)
                    for kc in range(NKC - 3, NKC):
                        emit_pv(kc)

                    # normalize: D replicated on pv partitions 0:64, PV on 64:128
                    bcsb = bc_pool.tile([64, 1024], f32, name="bc_sb", tag="bc_sb")
                    for h in range(2):
                        nc.vector.reciprocal_approx_fast(
                            out=bcsb[:, h * 512:(h + 1) * 512],
                            in_=pv[h][0:64, :])
                    qcs = slice(qc * 512, (qc + 1) * 512)
                    nc.vector.tensor_mul(
                        attnT[b][0:64, qcs], pv[0][64:128, :], bcsb[:, 0:512])
                    nc.vector.tensor_mul(
                        attnT[b][64:CH, qcs], pv[1][64:128, :], bcsb[:, 512:1024])

                    pending_proj += [(b, tci) for tci in range(qc * 4, (qc + 1) * 4)]

            # flush the tail proj (last qc) across all DMA queues
            for b, tci in pending_proj:
                emit_proj_tc(b, tci, last=True)

    nc.compile()
    return nc


def _prep_inputs(x, W_qkv, b_qkv, W_proj, b_proj):
    bf = ml_dtypes.bfloat16
    xT = np.ascontiguousarray(
        x.reshape(NTOK, C).T).astype(bf)
    in_maps = []
    for c in range(NCORES):
        cs = slice(c * CH, (c + 1) * CH)
        wq = np.ascontiguousarray(
            (W_qkv[:, c * CH:(c + 1) * CH] * SCALE)
            .reshape(8, 128, CH).transpose(1, 0, 2).reshape(128, C)).astype(bf)
        wk = np.ascontiguousarray(
            W_qkv[:, C + c * CH:C + (c + 1) * CH]
            .reshape(8, 128, CH).transpose(1, 0, 2).reshape(128, C)).astype(bf)
        wv = np.ascontiguousarray(
            W_qkv[:, 2 * C + c * CH:2 * C + (c + 1) * CH]
            .reshape(8, 128, CH).transpose(1, 0, 2).reshape(128, C)).astype(bf)
        wp = np.ascontiguousarray(W_proj[cs, :]).astype(bf)
        bqc = (b_qkv[c * CH:(c + 1) * CH] * SCALE).reshape(CH, 1).astype(np.float32)
        bkc = b_qkv[C + c * CH:C + (c + 1) * CH].reshape(CH, 1).astype(np.float32)
        in_maps.append({
            "xT": xT, "wq": wq, "wk": wk, "wv": wv, "wp": wp,
            "bqc": bqc, "bkc": bkc,
        })
    return in_maps


def _run(inputs, trace=False):
    from concourse import bass_utils
    if "nc" not in _CACHE:
        _CACHE["nc"] = _build()
    nc = _CACHE["nc"]
    x = np.asarray(inputs["x"], np.float32)
    W_qkv = np.asarray(inputs["W_qkv"], np.float32)
    b_qkv = np.asarray(inputs["b_qkv"], np.float32)
    W_proj = np.asarray(inputs["W_proj"], np.float32)
    b_proj = np.asarray(inputs["b_proj"], np.float32)
    in_maps = _prep_inputs(x, W_qkv, b_qkv, W_proj, b_proj)
    br = bass_utils.run_bass_kernel_spmd(
        nc, in_maps, core_ids=list(range(NCORES)), trace=trace)
    partial = np.zeros((NTOK, C), np.float64)
    for r in br.results:
        partial += np.asarray(r["out"]).astype(np.float64)
    # V bias passes through softmax (weights sum to 1) -> constant bv@Wp
    bias = b_proj.astype(np.float64) + (
        b_qkv[2 * C:].astype(np.float64) @ W_proj.astype(np.float64))
    out = (partial + bias[None, :]).astype(np.float32).reshape(B, T, C)
    return out, br


def kernel(**inputs) -> np.ndarray:
    out, _ = _run(inputs, trace=False)
    return out
